# revision 1
# baseline (speedup 1.0000x reference)
"""Trainium2 Bass kernel for CausalSelfAttention with sliding-window + sink mask.

Sharding: 8 cores = (batch 2) x (sequence chunks of 512). Each core computes
QKV (+RoPE) for its 512 queries and for a kv range [4 sink | 256 halo |
512 own | 124 zero-pad] = 896 positions, runs banded attention in a
scores-transposed [k, q] layout (7 key-chunks of 128 with fixed q-windows,
multiplicative post-exp masking, denominator via a ones-column in V), then
projects with w_proj emitting a transposed [C, 512] output that the host
re-transposes and concatenates.

All matmuls run as float32r (full-rate fp32 path of the PE).
"""

import numpy as np

B, T, C, NH, HD = 2, 2048, 1024, 16, 64
WIN, SINK = 256, 4
CH = 512          # queries per core
KV = 896          # 512 own + 256 halo + 4 sink + 124 pad
NCORES = 8
W_C = [384, 512, 256, 256, 128, 256, 512]
OFF_C = [0, 0, 256, 256, 0, 0, 0]
MOFF = np.concatenate([[0], np.cumsum(W_C)]).astype(int)  # mask col offsets
MTOT = int(MOFF[-1])  # 1924

_cache = {}


def _build_nc():
    import concourse.bacc as bacc
    import concourse.mybir as mybir
    import concourse.tile as tile

    f32 = mybir.dt.float32
    f32r = mybir.dt.float32r
    AF = mybir.ActivationFunctionType

    nc = bacc.Bacc("TRN2", target_bir_lowering=False, debug=False,
                   num_devices=NCORES)

    xT = nc.dram_tensor("xT", [C, KV], f32r, kind="ExternalInput").ap()
    wqs = nc.dram_tensor("wqs", [C, C], f32r, kind="ExternalInput").ap()
    wks = nc.dram_tensor("wks", [C, C], f32r, kind="ExternalInput").ap()
    wv = nc.dram_tensor("wv", [C, C], f32r, kind="ExternalInput").ap()
    wps = nc.dram_tensor("wps", [C, C], f32r, kind="ExternalInput").ap()
    cos_q = nc.dram_tensor("cos_q", [128, CH], f32, kind="ExternalInput").ap()
    sin_q = nc.dram_tensor("sin_q", [128, CH], f32, kind="ExternalInput").ap()
    cos_k = nc.dram_tensor("cos_k", [128, KV], f32, kind="ExternalInput").ap()
    sin_k = nc.dram_tensor("sin_k", [128, KV], f32, kind="ExternalInput").ap()
    masks = nc.dram_tensor("masks", [128, MTOT], f32, kind="ExternalInput").ap()
    p2d = nc.dram_tensor("p2", [128, 128], f32r, kind="ExternalInput").ap()
    rseld = nc.dram_tensor("rsel", [16, C], f32r, kind="ExternalInput").ap()
    onesd = nc.dram_tensor("ones", [128, 16], f32, kind="ExternalInput").ap()
    outT = nc.dram_tensor("outT", [C, CH], f32, kind="ExternalOutput").ap()

    KSEG = [(0, 512), (512, 384)]  # kv free-dim segments (psum bank limit)

    with tile.TileContext(nc) as tc:
        with (
            tc.tile_pool(name="pers", bufs=1) as pers,
            tc.tile_pool(name="wsl", bufs=2) as wsl,
            tc.tile_pool(name="big", bufs=8) as big,     # wv chunks then praw/outT
            tc.tile_pool(name="qk", bufs=2) as qkp,
            tc.tile_pool(name="tmp", bufs=2) as tmp,
            tc.tile_pool(name="yts", bufs=1) as ytsp,
            tc.tile_pool(name="ptp", bufs=14) as ptp,
            tc.tile_pool(name="sm", bufs=2) as smp,
            tc.tile_pool(name="psmm", bufs=2, space="PSUM") as psmm,
            tc.tile_pool(name="pssc", bufs=4, space="PSUM") as pssc,
            tc.tile_pool(name="psyt", bufs=2, space="PSUM") as psyt,
        ):
            # ---------- persistent loads ----------
            xa, xb = [], []
            for i in range(8):
                t = pers.tile([128, 512], f32r, tag=f"xa{i}", name=f"xa{i}")
                nc.sync.dma_start(t[:], xT[i * 128:(i + 1) * 128, 0:512])
                xa.append(t)
                t = pers.tile([128, 384], f32r, tag=f"xb{i}", name=f"xb{i}")
                nc.sync.dma_start(t[:], xT[i * 128:(i + 1) * 128, 512:896])
                xb.append(t)
            tcos_q = pers.tile([128, CH], f32, tag="cos_q")
            nc.sync.dma_start(tcos_q[:], cos_q[:])
            tsin_q = pers.tile([128, CH], f32, tag="sin_q")
            nc.sync.dma_start(tsin_q[:], sin_q[:])
            tcos_k = pers.tile([128, KV], f32, tag="cos_k")
            nc.sync.dma_start(tcos_k[:], cos_k[:])
            tsin_k = pers.tile([128, KV], f32, tag="sin_k")
            nc.sync.dma_start(tsin_k[:], sin_k[:])
            tmask = pers.tile([128, MTOT], f32, tag="mask")
            nc.sync.dma_start(tmask[:], masks[:])
            tp2 = pers.tile([128, 128], f32r, tag="p2")
            nc.sync.dma_start(tp2[:], p2d[:])
            trsel = pers.tile([16, C], f32r, tag="rsel")
            nc.sync.dma_start(trsel[:], rseld[:])
            tones = pers.tile([128, 16], f32, tag="ones")
            nc.sync.dma_start(tones[:], onesd[:])

            # ---------- V = xT.T @ wv in [k, d] layout with ones columns ----------
            wvc = []
            for kc in range(8):
                t = big.tile([128, 1024], f32r, tag="big", name=f"wvc{kc}")
                nc.sync.dma_start(t[:], wv[kc * 128:(kc + 1) * 128, :])
                wvc.append(t)
            v_sb = []
            for tt in range(7):
                vt = pers.tile([128, 1040], f32r, tag=f"v{tt}", name=f"v{tt}")
                vr = vt.rearrange("p (h e) -> p h e", e=65)
                pv = [psmm.tile([128, 512], f32, tag="mm", name=f"pv{tt}_{i}")
                      for i in range(2)]
                for kc in range(8):
                    xsl = (xa[kc][:, tt * 128:(tt + 1) * 128] if tt < 4 else
                           xb[kc][:, (tt - 4) * 128:(tt - 3) * 128])
                    for dh in range(2):
                        nc.tensor.matmul(
                            pv[dh][:], xsl,
                            wvc[kc][:, dh * 512:(dh + 1) * 512],
                            start=(kc == 0), stop=(kc == 7),
                        )
                for dh in range(2):
                    nc.scalar.copy(
                        vr[:, dh * 8:(dh + 1) * 8, 0:64],
                        pv[dh][:].rearrange("p (h e) -> p h e", e=64),
                    )
                nc.scalar.copy(vr[:, :, 64:65],
                               tones[:].rearrange("p (h o) -> p h o", o=1))
                v_sb.append(vt)

            d16 = smp.tile([16, 512], f32, tag="d16")
            AVORD = [1, 6, 0, 5, 4, 2, 3]

            def qkv_rope(hp):
                # qT raw
                wq_sl = wsl.tile([128, 1024], f32r, tag="wslab",
                                 name=f"wq{hp}")
                nc.sync.dma_start(wq_sl[:], wqs[hp * 128:(hp + 1) * 128, :])
                pq = psmm.tile([128, 512], f32, tag="mm", name=f"pq{hp}")
                for kc in range(8):
                    nc.tensor.matmul(
                        pq[:], wq_sl[:, kc * 128:(kc + 1) * 128],
                        xa[kc][:],
                        start=(kc == 0), stop=(kc == 7),
                    )
                qraw = tmp.tile([128, CH], f32r, tag="qraw", name=f"qraw{hp}")
                nc.scalar.copy(qraw[:], pq[:])

                # kT raw (segments share each weight load)
                wk_sl = wsl.tile([128, 1024], f32r, tag="wslab",
                                 name=f"wk{hp}")
                nc.sync.dma_start(wk_sl[:], wks[hp * 128:(hp + 1) * 128, :])
                kraw = tmp.tile([128, KV], f32r, tag="kraw", name=f"kraw{hp}")
                pk = [psmm.tile([128, 512], f32, tag="mm", name=f"pk{hp}_{i}")
                      for i in range(2)]
                for kc in range(8):
                    for si, (s0, sw) in enumerate(KSEG):
                        rhs = xa[kc][:] if si == 0 else xb[kc][:]
                        nc.tensor.matmul(
                            pk[si][:, 0:sw], wk_sl[:, kc * 128:(kc + 1) * 128],
                            rhs, start=(kc == 0), stop=(kc == 7),
                        )
                for si, (s0, sw) in enumerate(KSEG):
                    nc.scalar.copy(kraw[:, s0:s0 + sw], pk[si][:, 0:sw])

                # rope
                qT = qkp.tile([128, CH], f32r, tag="qT", name=f"qT{hp}")
                prot = psmm.tile([128, 512], f32, tag="mm", name=f"prot{hp}")
                nc.tensor.matmul(prot[:], tp2[:], qraw[:], start=True, stop=True)
                t2 = tmp.tile([128, CH], f32, tag="t2", name=f"t2q{hp}")
                nc.vector.tensor_mul(t2[:], prot[:], tsin_q[:])
                nc.vector.tensor_mul(qraw[:], qraw[:], tcos_q[:])
                nc.vector.tensor_add(qT[:], qraw[:], t2[:])

                kT = qkp.tile([128, KV], f32r, tag="kT", name=f"kT{hp}")
                for si, (s0, sw) in enumerate(KSEG):
                    prk = psmm.tile([128, 512], f32, tag="mm",
                                    name=f"prk{hp}_{si}")
                    nc.tensor.matmul(prk[:, 0:sw], tp2[:],
                                     kraw[:, s0:s0 + sw], start=True, stop=True)
                    t2k = tmp.tile([128, 512], f32, tag="t2",
                                   name=f"t2k{hp}_{si}")
                    nc.vector.tensor_mul(t2k[:, 0:sw], prk[:, 0:sw],
                                         tsin_k[:, s0:s0 + sw])
                    nc.vector.tensor_mul(kraw[:, s0:s0 + sw],
                                         kraw[:, s0:s0 + sw],
                                         tcos_k[:, s0:s0 + sw])
                    nc.vector.tensor_add(kT[:, s0:s0 + sw],
                                         kraw[:, s0:s0 + sw], t2k[:, 0:sw])
                return qT, kT

            def sc_block(hp, qT, kT):
                # scoresT matmuls issued as adjacent row-tile pairs (K=64 at
                # partition bases 0/64 -> concurrent in the PE array), then
                # exp (psum->sbuf, fused 1/sqrt(hd) scale) and mask multiply.
                pts = {}
                for c in range(7):
                    w, off = W_C[c], OFF_C[c]
                    scs = []
                    for half in range(2):
                        dsl = slice(half * 64, half * 64 + 64)
                        sc = pssc.tile([128, 512], f32, tag="sc",
                                       name=f"sc{hp}_{c}_{half}")
                        nc.tensor.matmul(
                            sc[:, 0:w], kT[dsl, c * 128:(c + 1) * 128],
                            qT[dsl, off:off + w], start=True, stop=True,
                        )
                        scs.append(sc)
                    for half in range(2):
                        w, off = W_C[c], OFF_C[c]
                        praw = big.tile([128, 512], f32r, tag="big",
                                        name=f"praw{hp}_{c}_{half}")
                        nc.scalar.activation(praw[:, 0:w], scs[half][:, 0:w],
                                             AF.Exp, scale=0.125)
                        pt = ptp.tile([128, 512], f32r, tag="pt",
                                      name=f"pt{hp}_{c}_{half}")
                        nc.vector.tensor_mul(
                            pt[:, 0:w], praw[:, 0:w],
                            tmask[:, MOFF[c]:MOFF[c] + w],
                        )
                        pts[(c, half)] = pt
                return pts

            def av_block(hp, pts):
                yt_pair = []
                for half in range(2):
                    h = hp * 2 + half
                    yt = psyt.tile([65, 512], f32, tag="yt",
                                   name=f"yt{hp}_{half}")
                    for ci, c in enumerate(AVORD):
                        w, off = W_C[c], OFF_C[c]
                        nc.tensor.matmul(
                            yt[:, off:off + w],
                            v_sb[c][:, h * 65:(h + 1) * 65],
                            pts[(c, half)][:, 0:w],
                            start=(ci == 0), stop=(ci == 6),
                        )
                    yt_pair.append(yt)
                ytu = ytsp.tile([128, CH], f32r, tag=f"ytu{hp}",
                                name=f"ytu{hp}")
                nc.scalar.copy(ytu[0:64, :], yt_pair[0][0:64, :])
                nc.scalar.copy(ytu[64:128, :], yt_pair[1][0:64, :])
                for half in range(2):
                    dt_ = smp.tile([1, 512], f32, tag="dt",
                                   name=f"dt{hp}_{half}")
                    nc.scalar.copy(dt_[:], yt_pair[half][64:65, :])
                    nc.sync.dma_start(
                        d16[2 * hp + half:2 * hp + half + 1, :], dt_[:])
                return ytu

            # ---------- software-pipelined head-pair loop ----------
            yts = []
            qk_state = qkv_rope(0)
            for hp in range(8):
                pts = sc_block(hp, *qk_state)
                if hp < 7:
                    qk_state = qkv_rope(hp + 1)
                yts.append(av_block(hp, pts))

            # ---------- batched normalization (in place, rounds to f32r) ----
            r16 = smp.tile([16, 512], f32r, tag="r16")
            with nc.allow_low_precision(reason="f32r recip for PE broadcast"):
                nc.vector.reciprocal(r16[:], d16[:])
            for hp in range(8):
                prb = psmm.tile([128, 512], f32, tag="mm", name=f"prb{hp}")
                nc.tensor.matmul(prb[:], trsel[:, hp * 128:(hp + 1) * 128],
                                 r16[:], start=True, stop=True)
                nc.vector.tensor_mul(yts[hp][0:64, :], yts[hp][0:64, :],
                                     prb[0:64, :])
                nc.vector.tensor_mul(yts[hp][64:128, :], yts[hp][64:128, :],
                                     prb[64:128, :])

            # ---------- projection (transposed output) ----------
            for cc in range(8):
                wp_sl = wsl.tile([128, 1024], f32r, tag="wslab",
                                 name=f"wp{cc}")
                nc.sync.dma_start(wp_sl[:], wps[cc * 128:(cc + 1) * 128, :])
                po = psmm.tile([128, 512], f32, tag="mm", name=f"po{cc}")
                for hp in range(8):
                    nc.tensor.matmul(
                        po[:], wp_sl[:, hp * 128:(hp + 1) * 128], yts[hp][:],
                        start=(hp == 0), stop=(hp == 7),
                    )
                osb = big.tile([128, 512], f32, tag="big", name=f"osb{cc}")
                nc.scalar.copy(osb[:], po[:])
                nc.sync.dma_start(outT[cc * 128:(cc + 1) * 128, :], osb[:])

    nc.compile()
    return nc


def _host_inputs(x, w_attn, w_proj):
    """Build the 8 per-core input maps."""
    inv_freq = 1.0 / (10000.0 ** (np.arange(0, HD, 2, dtype=np.float32) / HD))
    iff = np.concatenate([inv_freq, inv_freq])  # [64]

    def cos_sin(pos):
        ang = pos[None, :].astype(np.float32) * iff[:, None]
        c = np.concatenate([np.cos(ang), np.cos(ang)], 0).astype(np.float32)
        s = np.concatenate([np.sin(ang), np.sin(ang)], 0).astype(np.float32)
        return np.ascontiguousarray(c), np.ascontiguousarray(s)

    P2 = np.zeros((128, 128), np.float32)
    for blk in range(2):
        o = blk * 64
        for d in range(32):
            P2[o + d + 32, o + d] = -1.0
            P2[o + d, o + d + 32] = 1.0

    rsel = np.zeros((16, C), np.float32)
    for h in range(16):
        hp, half = h // 2, h % 2
        rsel[h, hp * 128 + half * 64: hp * 128 + half * 64 + 64] = 1.0
    ones16 = np.ones((128, 16), np.float32)

    def shuffle_lhsT(w):
        # rows (kc*128 + c_lo), cols (hp*128 + d) ->
        # rows (hp*128 + c_lo), cols (kc*128 + d)
        return np.ascontiguousarray(
            w.reshape(8, 128, 8, 128).transpose(2, 1, 0, 3).reshape(C, C)
        )

    wq = shuffle_lhsT(w_attn[:, 0:C])
    wk = shuffle_lhsT(w_attn[:, C:2 * C])
    wvm = np.ascontiguousarray(w_attn[:, 2 * C:3 * C])
    wp = shuffle_lhsT(w_proj)

    in_maps = []
    for core in range(NCORES):
        b, j = core // 4, core % 4
        q0 = j * CH
        kv_gk = np.full(KV, -1, np.int64)
        kv_gk[0:512] = q0 + np.arange(CH)
        halo = q0 - 256 + np.arange(256)
        kv_gk[512:768] = np.where(halo >= 0, halo, -1)
        kv_gk[768:772] = np.arange(4)

        xTc = np.zeros((C, KV), np.float32)
        valid = kv_gk >= 0
        xTc[:, valid] = x[b, kv_gk[valid]].T

        cq, sq = cos_sin(q0 + np.arange(CH))
        ck, sk = cos_sin(np.maximum(kv_gk, 0))

        gq = q0 + np.arange(CH)
        mask = np.zeros((128, MTOT), np.float32)
        for c in range(7):
            rows = c * 128 + np.arange(128)
            gk = kv_gk[rows]
            qw = gq[OFF_C[c]:OFF_C[c] + W_C[c]]
            real = (rows < 772) & (gk >= 0)
            g = np.where(real, gk, 0)[:, None]
            qq = qw[None, :]
            is_sink = ((rows >= 768) & (rows < 772))[:, None]
            allow = np.where(
                is_sink,
                (g <= qq) & (qq - g >= WIN),
                (g <= qq) & (qq - g < WIN),
            )
            allow &= real[:, None]
            mask[:, MOFF[c]:MOFF[c] + W_C[c]] = allow.astype(np.float32)

        in_maps.append({
            "xT": xTc, "wqs": wq, "wks": wk, "wv": wvm, "wps": wp,
            "cos_q": cq, "sin_q": sq, "cos_k": ck, "sin_k": sk,
            "masks": mask, "p2": P2, "rsel": rsel, "ones": ones16,
        })
    return in_maps


def kernel(x, w_attn, w_proj):
    from concourse import bass_utils

    x = np.asarray(x, np.float32)
    w_attn = np.asarray(w_attn, np.float32)
    w_proj = np.asarray(w_proj, np.float32)

    if "nc" not in _cache:
        _cache["nc"] = _build_nc()
    nc = _cache["nc"]

    in_maps = _host_inputs(x, w_attn, w_proj)
    res = bass_utils.run_bass_kernel_spmd(nc, in_maps, list(range(NCORES)),
                                          **_cache.get("run_kwargs", {}))
    _cache["last_result"] = res

    y = np.zeros((B, T, C), np.float32)
    for core in range(NCORES):
        b, j = core // 4, core % 4
        y[b, j * CH:(j + 1) * CH, :] = res.results[core]["outT"].T
    return y



# revision 9
# speedup vs baseline: 1.6426x; 1.6426x over previous
"""Trainium2 Bass kernel for CausalSelfAttention with sliding-window + sink mask.

Sharding: 8 cores = (batch 2) x (sequence chunks of 512). Each core computes
QKV (+RoPE) for its 512 queries and a tight kv range [512 own | 256 halo |
4 sink] = 772 positions, runs banded attention in a scores-transposed [k, q]
layout with chunks packed into two [128, 1024] PSUM supertiles per head-half
(multiplicative post-exp masking, denominator via a ones-column in V), then
projects with w_proj emitting a transposed [C, 512] output that the host
re-transposes and concatenates.

v2: bf16 operands everywhere (weights, x, attention internals; fp32 PSUM),
packed score tiles to halve exp/mask instruction count, engine
load-balancing (exp on scalar, rope/mask/norm on vector, copies on gpsimd),
and an interleaved PE schedule that keeps the tensor engine dense so the
HAM clock stays at 2.4 GHz.
"""

import numpy as np

B, T, C, NH, HD = 2, 2048, 1024, 16, 64
WIN, SINK = 256, 4
CH = 512          # queries per core
KV = 772          # 512 own + 256 halo + 4 sink
KVP = 896         # padded key space for 7x128 score chunking
NCORES = 8

# Packed score-tile layout: (tile_idx, col_off, key_chunk, W, q_off).
# Key chunks: 0-3 own kv[0:512], 4 halo-lo kv[512:640], 5 halo-hi
# kv[640:768], 6 sink kv[768:772]+pad. Each supertile is [128, 1024]
# (2 PSUM banks); no matmul write crosses a 512-col bank boundary.
CHUNKS = [
    (0, 0,   0, 384, 0),
    (0, 384, 3, 128, 384),
    (0, 512, 1, 384, 128),
    (0, 896, 4, 128, 0),
    (1, 0,   2, 256, 256),
    (1, 256, 5, 256, 0),
    (1, 512, 6, 512, 0),
]
# AV issue order: the first two (c0 q[0:384), c3 q[384:512)) carry
# start=True and together initialize every yt column before any
# accumulating matmul touches it.
MTOT = 2048

_cache = {}


def _build_nc():
    import concourse.bacc as bacc
    import concourse.mybir as mybir
    import concourse.tile as tile

    f32 = mybir.dt.float32
    bf16 = mybir.dt.bfloat16
    AF = mybir.ActivationFunctionType

    nc = bacc.Bacc("TRN2", target_bir_lowering=False, debug=False,
                   num_devices=NCORES)

    xTd = nc.dram_tensor("xT", [C, KV], bf16, kind="ExternalInput").ap()
    wqd = nc.dram_tensor("wqs", [C, C], bf16, kind="ExternalInput").ap()
    wkd = nc.dram_tensor("wks", [C, C], bf16, kind="ExternalInput").ap()
    wvd = nc.dram_tensor("wv", [C, C], bf16, kind="ExternalInput").ap()
    wpd = nc.dram_tensor("wps", [C, C], bf16, kind="ExternalInput").ap()
    cqd = nc.dram_tensor("cos_q", [128, CH], bf16, kind="ExternalInput").ap()
    sqd = nc.dram_tensor("sin_q", [128, CH], bf16, kind="ExternalInput").ap()
    ckd = nc.dram_tensor("cos_k", [128, KV], bf16, kind="ExternalInput").ap()
    skd = nc.dram_tensor("sin_k", [128, KV], bf16, kind="ExternalInput").ap()
    maskd = nc.dram_tensor("masks", [128, MTOT], bf16, kind="ExternalInput").ap()
    p2d = nc.dram_tensor("p2", [128, 128], bf16, kind="ExternalInput").ap()
    rseld = nc.dram_tensor("rsel", [16, C], bf16, kind="ExternalInput").ap()
    onesd = nc.dram_tensor("ones", [128, 16], bf16, kind="ExternalInput").ap()
    outT = nc.dram_tensor("outT", [C, CH], f32, kind="ExternalOutput").ap()

    with tile.TileContext(nc) as tc:
        with (
            tc.tile_pool(name="pers", bufs=1) as pers,
            tc.tile_pool(name="tmp", bufs=2) as tmp,
            tc.tile_pool(name="tmp2", bufs=2) as tmp2,
            tc.tile_pool(name="qk", bufs=2) as qkp,
            tc.tile_pool(name="ptp", bufs=8) as ptp,
            tc.tile_pool(name="big", bufs=2) as big,
            tc.tile_pool(name="psk", bufs=1, space="PSUM") as psk,
            tc.tile_pool(name="pssc", bufs=2, space="PSUM") as pssc,
            tc.tile_pool(name="psmm", bufs=2, space="PSUM") as psmm,
        ):
            # ---------- persistent loads (priority order) ----------
            xab = []
            for i in range(8):
                t = pers.tile([128, KV], bf16, tag=f"xab{i}", name=f"xab{i}")
                nc.sync.dma_start(t[:], xTd[i * 128:(i + 1) * 128, :])
                xab.append(t)

            wq_t = [pers.tile([128, C], bf16, tag=f"wq{i}", name=f"wq{i}")
                    for i in range(8)]
            wk_t = [pers.tile([128, C], bf16, tag=f"wk{i}", name=f"wk{i}")
                    for i in range(8)]
            wv_t = [pers.tile([128, C], bf16, tag=f"wv{i}", name=f"wv{i}")
                    for i in range(8)]
            wp_t = [pers.tile([128, C], bf16, tag=f"wp{i}", name=f"wp{i}")
                    for i in range(8)]

            nc.sync.dma_start(wq_t[0][:], wqd[0:128, :])
            nc.sync.dma_start(wk_t[0][:], wkd[0:128, :])

            tp2 = pers.tile([128, 128], bf16, tag="p2")
            nc.sync.dma_start(tp2[:], p2d[:])
            tcos_q = pers.tile([128, CH], bf16, tag="cos_q")
            nc.sync.dma_start(tcos_q[:], cqd[:])
            tsin_q = pers.tile([128, CH], bf16, tag="sin_q")
            nc.sync.dma_start(tsin_q[:], sqd[:])
            tcos_k = pers.tile([128, KV], bf16, tag="cos_k")
            nc.sync.dma_start(tcos_k[:], ckd[:])
            tsin_k = pers.tile([128, KV], bf16, tag="sin_k")
            nc.sync.dma_start(tsin_k[:], skd[:])

            for i in range(8):
                nc.sync.dma_start(wv_t[i][:], wvd[i * 128:(i + 1) * 128, :])
            tmask = pers.tile([128, MTOT], bf16, tag="mask")
            nc.sync.dma_start(tmask[:], maskd[:])
            tones = pers.tile([128, 16], bf16, tag="ones")
            nc.sync.dma_start(tones[:], onesd[:])
            for i in range(1, 8):
                nc.sync.dma_start(wq_t[i][:], wqd[i * 128:(i + 1) * 128, :])
                nc.sync.dma_start(wk_t[i][:], wkd[i * 128:(i + 1) * 128, :])
            trsel = pers.tile([16, C], bf16, tag="rsel")
            nc.sync.dma_start(trsel[:], rseld[:])
            for i in range(8):
                nc.sync.dma_start(wp_t[i][:], wpd[i * 128:(i + 1) * 128, :])

            d16 = pers.tile([16, CH], f32, tag="d16")
            ytu = [pers.tile([128, CH], bf16, tag=f"ytu{i}", name=f"ytu{i}")
                   for i in range(8)]

            # ---------- rope/QKV halves (interleaved into the hp loop) ----
            def rope_q(hp):
                pq = psmm.tile([128, CH], f32, tag="mm", name=f"pq{hp}")
                for kc in range(8):
                    nc.tensor.matmul(
                        pq[:], wq_t[hp][:, kc * 128:(kc + 1) * 128],
                        xab[kc][:, 0:CH], start=(kc == 0), stop=(kc == 7),
                    )
                qraw = tmp.tile([128, CH], bf16, tag="qraw", name=f"qraw{hp}")
                nc.scalar.copy(qraw[:], pq[:])
                prot = psmm.tile([128, CH], f32, tag="mm", name=f"prot{hp}")
                nc.tensor.matmul(prot[:], tp2[:], qraw[:], start=True,
                                 stop=True)
                t2 = tmp2.tile([128, KV], bf16, tag="t2", name=f"t2q{hp}")
                nc.vector.tensor_mul(t2[:, 0:CH], prot[:], tsin_q[:])
                qc = tmp2.tile([128, KV], bf16, tag="tc", name=f"qc{hp}")
                nc.gpsimd.tensor_mul(qc[:, 0:CH], qraw[:], tcos_q[:])
                qT = qkp.tile([128, CH], bf16, tag="qT", name=f"qT{hp}")
                nc.vector.tensor_add(qT[:], qc[:, 0:CH], t2[:, 0:CH])
                return qT

            def rope_k(hp):
                pk = psk.tile([128, KV], f32, tag="kk", name=f"pk{hp}")
                for kc in range(8):
                    nc.tensor.matmul(
                        pk[:, 0:512], wk_t[hp][:, kc * 128:(kc + 1) * 128],
                        xab[kc][:, 0:512], start=(kc == 0), stop=(kc == 7),
                    )
                for kc in range(8):
                    nc.tensor.matmul(
                        pk[:, 512:KV], wk_t[hp][:, kc * 128:(kc + 1) * 128],
                        xab[kc][:, 512:KV], start=(kc == 0), stop=(kc == 7),
                    )
                kraw = tmp.tile([128, KV], bf16, tag="kraw", name=f"kraw{hp}")
                nc.vector.tensor_copy(kraw[:], pk[:])
                prk = psk.tile([128, KV], f32, tag="kk", name=f"prk{hp}")
                nc.tensor.matmul(prk[:, 0:512], tp2[:], kraw[:, 0:512],
                                 start=True, stop=True)
                nc.tensor.matmul(prk[:, 512:KV], tp2[:], kraw[:, 512:KV],
                                 start=True, stop=True)
                t2k = tmp2.tile([128, KV], bf16, tag="t2", name=f"t2k{hp}")
                nc.vector.tensor_mul(t2k[:], prk[:], tsin_k[:])
                kc_ = tmp2.tile([128, KV], bf16, tag="tc", name=f"kc{hp}")
                nc.gpsimd.tensor_mul(kc_[:], kraw[:], tcos_k[:])
                kT = qkp.tile([128, KVP], bf16, tag="kT", name=f"kT{hp}")
                nc.gpsimd.memset(kT[:, KV:KVP], 0.0)
                nc.vector.tensor_add(kT[:, 0:KV], kc_[:], t2k[:])
                return kT

            # ---------- V = xT.T @ wv in [k, d] layout with ones column ----
            qT0 = rope_q(0)
            kT0 = rope_k(0)

            v_sb = []
            for tt in range(7):
                vt = pers.tile([128, 1040], bf16, tag=f"v{tt}", name=f"v{tt}")
                vr = vt.rearrange("p (h e) -> p h e", e=65)
                if tt == 6:
                    nc.gpsimd.memset(vt[:], 0.0)
                pv = [psmm.tile([128, CH], f32, tag="mm", name=f"pv{tt}_{i}")
                      for i in range(2)]
                for kc in range(8):
                    if tt == 6:
                        xsl = xab[kc][:, 768:772]
                    else:
                        xsl = xab[kc][:, tt * 128:(tt + 1) * 128]
                    for dh in range(2):
                        nc.tensor.matmul(
                            pv[dh][:, 0:CH] if tt != 6 else pv[dh][0:4, 0:CH],
                            xsl, wv_t[kc][:, dh * 512:(dh + 1) * 512],
                            start=(kc == 0), stop=(kc == 7),
                        )
                nrow = 128 if tt != 6 else 4
                for dh in range(2):
                    nc.scalar.copy(
                        vr[0:nrow, dh * 8:(dh + 1) * 8, 0:64],
                        pv[dh][0:nrow].rearrange("p (h e) -> p h e", e=64),
                    )
                nc.scalar.copy(vr[:, :, 64:65],
                               tones[:].rearrange("p (h o) -> p h o", o=1))
                v_sb.append(vt)

            # ---------- attention blocks ----------
            def sc_half(hp, half, qT, kT):
                dsl = slice(half * 64, half * 64 + 64)
                sts = []
                for tidx in range(2):
                    st = pssc.tile([128, 1024], f32, tag="sc",
                                   name=f"st{hp}_{half}_{tidx}")
                    for (ti, coff, kc, w, qoff) in CHUNKS:
                        if ti != tidx:
                            continue
                        nc.tensor.matmul(
                            st[:, coff:coff + w],
                            kT[dsl, kc * 128:(kc + 1) * 128],
                            qT[dsl, qoff:qoff + w], start=True, stop=True,
                        )
                    sts.append(st)
                return sts

            def exp_mask(hp, half, sts):
                pts = []
                for tidx, st in enumerate(sts):
                    praw = ptp.tile([128, 1024], bf16, tag="pt",
                                    name=f"praw{hp}_{half}_{tidx}")
                    nc.scalar.activation(praw[:], st[:], AF.Exp, scale=0.125)
                    pt = ptp.tile([128, 1024], bf16, tag="pt",
                                  name=f"pt{hp}_{half}_{tidx}")
                    nc.vector.tensor_mul(
                        pt[:], praw[:],
                        tmask[:, tidx * 1024:(tidx + 1) * 1024])
                    pts.append(pt)
                return pts

            # AV order: sink chunk first — its full 512-col window
            # initializes every yt element (start=True), the rest accumulate.
            AV_CHUNKS = [CHUNKS[6]] + CHUNKS[0:6]

            def av_half(hp, half, pts):
                h = hp * 2 + half
                yt = psmm.tile([128, CH], f32, tag="mm",
                               name=f"yt{hp}_{half}")
                for ci, (ti, coff, kc, w, qoff) in enumerate(AV_CHUNKS):
                    nc.tensor.matmul(
                        yt[0:65, qoff:qoff + w],
                        v_sb[kc][:, h * 65:(h + 1) * 65],
                        pts[ti][:, coff:coff + w],
                        start=(ci == 0), stop=(ci == 6),
                        skip_group_check=True,
                    )
                # stage denominator + numerator out of PSUM (engines cannot
                # write at a partition offset, so bounce the denom via DMA)
                dt_ = tmp.tile([1, CH], f32, tag="dt", name=f"dt{hp}_{half}")
                nc.scalar.copy(dt_[:], yt[64:65, :])
                nc.sync.dma_start(d16[h:h + 1, :], dt_[:])
                nc.scalar.copy(ytu[hp][half * 64:half * 64 + 64, :],
                               yt[0:64, :])

            # ---------- software-pipelined head-pair loop ----------
            qT, kT = qT0, kT0
            for hp in range(8):
                sts0 = sc_half(hp, 0, qT, kT)
                pts0 = exp_mask(hp, 0, sts0)
                nqT = rope_q(hp + 1) if hp < 7 else None
                sts1 = sc_half(hp, 1, qT, kT)
                pts1 = exp_mask(hp, 1, sts1)
                nkT = rope_k(hp + 1) if hp < 7 else None
                av_half(hp, 0, pts0)
                av_half(hp, 1, pts1)
                qT, kT = nqT, nkT

            # ---------- batched normalization ----------
            r16f = pers.tile([16, CH], f32, tag="r16f")
            nc.vector.reciprocal_approx_fast(r16f[:], d16[:])
            r16 = pers.tile([16, CH], bf16, tag="r16")
            nc.vector.tensor_copy(r16[:], r16f[:])
            for hp in range(8):
                prb = psmm.tile([128, CH], f32, tag="mm", name=f"prb{hp}")
                nc.tensor.matmul(prb[:], trsel[:, hp * 128:(hp + 1) * 128],
                                 r16[:], start=True, stop=True)
                nc.vector.tensor_mul(ytu[hp][:], ytu[hp][:], prb[:])

            # ---------- projection (transposed output) ----------
            for cc in range(8):
                po = psmm.tile([128, CH], f32, tag="mm", name=f"po{cc}")
                for hp in range(8):
                    nc.tensor.matmul(
                        po[:], wp_t[cc][:, hp * 128:(hp + 1) * 128],
                        ytu[hp][:], start=(hp == 0), stop=(hp == 7),
                    )
                osb = big.tile([128, CH], f32, tag="osb", name=f"osb{cc}")
                nc.scalar.copy(osb[:], po[:])
                nc.sync.dma_start(outT[cc * 128:(cc + 1) * 128, :], osb[:])

    nc.compile()
    return nc


def _host_inputs(x, w_attn, w_proj):
    """Build the 8 per-core input maps (bf16 operands)."""
    import ml_dtypes
    bf = ml_dtypes.bfloat16

    inv_freq = 1.0 / (10000.0 ** (np.arange(0, HD, 2, dtype=np.float32) / HD))
    iff = np.concatenate([inv_freq, inv_freq])  # [64]

    def cos_sin(pos):
        ang = pos[None, :].astype(np.float32) * iff[:, None]
        c = np.concatenate([np.cos(ang), np.cos(ang)], 0)
        s = np.concatenate([np.sin(ang), np.sin(ang)], 0)
        return (np.ascontiguousarray(c.astype(bf)),
                np.ascontiguousarray(s.astype(bf)))

    P2 = np.zeros((128, 128), np.float32)
    for blk in range(2):
        o = blk * 64
        for d in range(32):
            P2[o + d + 32, o + d] = -1.0
            P2[o + d, o + d + 32] = 1.0

    rsel = np.zeros((16, C), np.float32)
    for h in range(16):
        hp, half = h // 2, h % 2
        rsel[h, hp * 128 + half * 64: hp * 128 + half * 64 + 64] = 1.0
    ones16 = np.ones((128, 16), np.float32)

    def shuffle_lhsT(w):
        # rows (kc*128 + c_lo), cols (hp*128 + d) ->
        # rows (hp*128 + c_lo), cols (kc*128 + d)
        return np.ascontiguousarray(
            w.reshape(8, 128, 8, 128).transpose(2, 1, 0, 3).reshape(C, C)
        )

    wq = shuffle_lhsT(w_attn[:, 0:C]).astype(bf)
    wk = shuffle_lhsT(w_attn[:, C:2 * C]).astype(bf)
    wvm = np.ascontiguousarray(w_attn[:, 2 * C:3 * C]).astype(bf)
    wp = shuffle_lhsT(w_proj).astype(bf)
    P2 = P2.astype(bf)
    rsel = rsel.astype(bf)
    ones16 = ones16.astype(bf)

    in_maps = []
    for core in range(NCORES):
        b, j = core // 4, core % 4
        q0 = j * CH
        # kv layout: [512 own | 256 halo | 4 sink]
        kv_gk = np.full(KV, -1, np.int64)
        kv_gk[0:512] = q0 + np.arange(CH)
        halo = q0 - 256 + np.arange(256)
        kv_gk[512:768] = np.where(halo >= 0, halo, -1)
        kv_gk[768:772] = np.arange(4)

        xTc = np.zeros((C, KV), np.float32)
        valid = kv_gk >= 0
        xTc[:, valid] = x[b, kv_gk[valid]].T

        cq, sq = cos_sin(q0 + np.arange(CH))
        ck, sk = cos_sin(np.maximum(kv_gk, 0))

        mask = np.zeros((128, MTOT), np.float32)
        for (tidx, coff, kc, w, qoff) in CHUNKS:
            p = np.arange(128)
            if kc < 4:
                g = q0 + kc * 128 + p
                real = np.ones(128, bool)
            elif kc in (4, 5):
                g = q0 - 256 + (kc - 4) * 128 + p
                real = g >= 0
            else:
                g = p.copy()
                real = p < 4
            gq = q0 + qoff + np.arange(w)
            gcol = np.where(real, g, 0)[:, None]
            qq = gq[None, :]
            if kc == 6:
                allow = (gcol <= qq) & (qq - gcol >= WIN)
            else:
                allow = (gcol <= qq) & (qq - gcol < WIN)
            allow &= real[:, None]
            mask[:, tidx * 1024 + coff: tidx * 1024 + coff + w] = \
                allow.astype(np.float32)

        in_maps.append({
            "xT": xTc.astype(bf), "wqs": wq, "wks": wk, "wv": wvm, "wps": wp,
            "cos_q": cq, "sin_q": sq, "cos_k": ck, "sin_k": sk,
            "masks": mask.astype(bf), "p2": P2, "rsel": rsel, "ones": ones16,
        })
    return in_maps


def kernel(x, w_attn, w_proj):
    from concourse import bass_utils

    x = np.asarray(x, np.float32)
    w_attn = np.asarray(w_attn, np.float32)
    w_proj = np.asarray(w_proj, np.float32)

    if "nc" not in _cache:
        _cache["nc"] = _build_nc()
    nc = _cache["nc"]

    in_maps = _host_inputs(x, w_attn, w_proj)
    res = bass_utils.run_bass_kernel_spmd(nc, in_maps, list(range(NCORES)),
                                          **_cache.get("run_kwargs", {}))
    _cache["last_result"] = res

    y = np.zeros((B, T, C), np.float32)
    for core in range(NCORES):
        b, j = core // 4, core % 4
        y[b, j * CH:(j + 1) * CH, :] = res.results[core]["outT"].T
    return y
